# revision 22
# baseline (speedup 1.0000x reference)
"""Trainium2 Bass kernel for nn_BertAttentionDistance (B=4, S=2048, H=1024, NH=1, DT=32).

Sharding: 8 cores = (batch b = c//2) x (query-half qh = c%2, 1024 q-rows each).
K/V projection work for a batch is replicated across its 2 cores (no collectives).
Key order per core is [own 1024 keys, other 1024 keys]; relT/am are permuted to
match on the host (softmax/PV are order-invariant over keys).

Math notes (exact identities w.r.t. the reference):
  * take_along_axis(word_dot_distance, rel, 3) * (rel == 1)
      == (q . dist_emb[1]) * (rel == 1)           (gather collapses)
  * wdd1 = q . d1 = hs . (Wq^T d1) + bq . d1 = hs . u + cq  (u, cq on host)
  * softmax max-subtraction skipped: scores/32 is O(+-3), safely in fp32 exp range.
  * v-bias and o-bias fold into the residual: x = Wo@ctx + (Wo@bv + bo) + hs,
      folded on host into hsq' = hs_q + Wo@bv + bo.
  * attention_mask folds into md = rel*wdd1/32 + am_k (general variant only).
  * ctxT stays UNNORMALIZED (pv/8 in fp8); 1/denominator is applied in the
      epilogue as a per-partition (per-query-row) scalar on the x-assembly,
      using denominators transposed via a DRAM bounce.

Engine split: PE all matmuls (fp8 DR); DVE psum evacuations (v, scores-stt,
pv, epi-stt) + LN stats; ACT k/q evacuations + exp + LN normalize; GPSIMD
md = rel*(wdd1/32) muls + rel DMA queue; sync queue big streams.
"""

import sys

for p in ("/opt/trn_rl_repo", "/opt/pypackages"):
    if p not in sys.path:
        sys.path.insert(0, p)

from contextlib import ExitStack

import ml_dtypes
import numpy as np

import concourse.bacc as bacc
import concourse.bass as bass
import concourse.tile as tile
from concourse import mybir
from concourse.bass_utils import run_bass_kernel_spmd

# Problem constants (hardcoded per the harness contract).
B, S, H = 4, 2048, 1024
N_CORES = 8
SQ = 1024          # query rows per core
HC = H // 128      # 8 chunks of 128 over hidden/head dim
KC = S // 128      # 16 chunks of 128 over key dim
KCH = KC // 2      # key chunks per half
LN_EPS = 1e-12
INV_SQRT_DH = 1.0 / 32.0

F32 = mybir.dt.float32
BF16 = mybir.dt.bfloat16
F8 = mybir.dt.float8e4
DR = mybir.MatmulPerfMode.DoubleRow
MULT = mybir.AluOpType.mult
ADD = mybir.AluOpType.add
AF = mybir.ActivationFunctionType

_CACHE = {}


def _build_program(am_zero=True, ln_affine=False):
    nc = bacc.Bacc("TRN2", target_bir_lowering=False, debug=False)

    # All big inputs are pre-swizzled on the host to exact SBUF tile layout
    # ([128 partitions, chunk, free]) so each DMA is 128 large descriptors.
    hsqT = nc.dram_tensor("hsqT", [128, HC, SQ], F8, kind="ExternalInput")
    hsoT = nc.dram_tensor("hsoT", [128, HC, SQ], F8, kind="ExternalInput")
    hsq = nc.dram_tensor("hsq", [128, HC, H], BF16, kind="ExternalInput")
    relT = nc.dram_tensor("relT", [S, SQ], F8, kind="ExternalInput")
    wqT = nc.dram_tensor("wqT", [128, HC, H], F8, kind="ExternalInput")
    wkT = nc.dram_tensor("wkT", [128, HC, H], F8, kind="ExternalInput")
    wvT = nc.dram_tensor("wvT", [128, HC, H], F8, kind="ExternalInput")
    woT = nc.dram_tensor("woT", [128, HC, H], F8, kind="ExternalInput")
    u_d = nc.dram_tensor("u", [128, HC, 64], F8, kind="ExternalInput")
    cq_d = nc.dram_tensor("cq32", [1, 1], F32, kind="ExternalInput")
    bq_d = nc.dram_tensor("bq", [128, HC], F32, kind="ExternalInput")
    bk_d = nc.dram_tensor("bk", [128, HC], F32, kind="ExternalInput")
    if not am_zero:
        am_d = nc.dram_tensor("am", [128, KC], F32, kind="ExternalInput")
    if ln_affine:
        lng_d = nc.dram_tensor("lng", [H], F32, kind="ExternalInput")
        lnb_d = nc.dram_tensor("lnb", [H], F32, kind="ExternalInput")
    den_d = nc.dram_tensor("den_scratch", [SQ], F32, kind="Internal")
    out_d = nc.dram_tensor("out", [SQ, H], BF16, kind="ExternalOutput")

    def bcast_rows(src_1d_ap, p=128):
        return bass.AP(
            tensor=src_1d_ap.tensor,
            offset=src_1d_ap.offset,
            ap=[[0, p], *src_1d_ap.ap],
        )

    with tile.TileContext(nc) as tc, ExitStack() as ctx:
        consts = ctx.enter_context(tc.tile_pool(name="consts", bufs=1))
        big = ctx.enter_context(tc.tile_pool(name="big", bufs=1))
        psum_sm = ctx.enter_context(tc.tile_pool(name="psum_sm", bufs=2, space="PSUM"))

        # DR ldweights needs >=64B stride between the two k-tiles: pad dim2.
        ones2_pad = consts.tile([128, 2, 64], F8)
        nc.vector.memset(ones2_pad, 1.0)
        ones2 = ones2_pad[:, :, 0:1]
        eps_t = consts.tile([128, 1], F32)
        nc.vector.memset(eps_t, LN_EPS)
        u_pad = consts.tile([128, HC, 64], F8)
        u_t = u_pad[:, :, 0:1]

        cq_t = consts.tile([1, 1], F32)
        bq_t = consts.tile([128, HC], F32)
        bk_t = consts.tile([128, HC], F32)
        if not am_zero:
            am_t = consts.tile([128, KC], F32)
            nc.gpsimd.dma_start(am_t, am_d[:])
        if ln_affine:
            g_b = consts.tile([128, H], F32)
            nc.gpsimd.dma_start(g_b, bcast_rows(lng_d[:]))
            b_b = consts.tile([128, H], F32)
            nc.gpsimd.dma_start(b_b, bcast_rows(lnb_d[:]))

        # ---- persistent big tensors ----
        kT = big.tile([128, HC, S], F8)        # [d-part, dc, k(own,other)]
        v_sb = big.tile([128, KC, H], F8)      # [s-part(own,other), kc, d]
        qT = big.tile([128, HC, SQ], F8)       # [d-part, dc, q]
        ctxT = big.tile([128, HC, SQ], F8)     # [d-part, dc, q] = pv/8
        md = big.tile([128, KC, SQ], BF16)     # rel*(wdd1/32) (+am) [k-part, kc, q]
        wdd_b = big.tile([128, SQ], BF16)      # broadcast of wdd1/32
        wdd_row = big.tile([1, SQ], BF16)
        expT = big.tile([128, KC, SQ], F8)     # exp (unnormalized) [k-part, kc, q]
        rinv = big.tile([128, HC], F32)        # 1/denom, [q-within-sc, sc]

        # ================= phase 1: projections + md =================
        with (
            tc.tile_pool(name="hs_pool", bufs=1) as hs_pool,
            tc.tile_pool(name="wpool", bufs=2) as wpool,
            tc.tile_pool(name="relp", bufs=3) as relp,
            tc.tile_pool(name="psum_p", bufs=3, space="PSUM") as psum_p,
        ):
            hsq_sb = hs_pool.tile([128, HC, SQ], F8)   # own rows (q==own keys)
            hso_sb = hs_pool.tile([128, HC, SQ], F8)   # other-half key rows

            wv_sb = wpool.tile([128, HC, H], F8, tag="w")

            # sync-queue DMA order = need order: u/cq first (wdd), then the
            # big streams; biases before the k/q evacuations need them.
            nc.sync.dma_start(u_pad, u_d[:])
            nc.sync.dma_start(cq_t, cq_d[:])
            nc.sync.dma_start(hsq_sb, hsqT[:])
            nc.sync.dma_start(wv_sb, wvT[:])
            nc.gpsimd.dma_start(hso_sb, hsoT[:])
            nc.sync.dma_start(bq_t, bq_d[:])
            nc.sync.dma_start(bk_t, bk_d[:])

            # ---- wdd1/32 row: M=1 DR matmuls from hsqT ----
            for qn in range(2):
                q_sl = slice(qn * 512, (qn + 1) * 512)
                ps1 = psum_sm.tile([1, 512], F32, tag="small")
                for dc in range(0, HC, 2):
                    nc.tensor.matmul(
                        ps1,
                        u_t[:, dc:dc + 2, :],
                        hsq_sb[:, dc:dc + 2, q_sl],
                        start=(dc == 0),
                        stop=(dc == HC - 2),
                        perf_mode=DR,
                    )
                # wdd1/32 = psum/(64*32) + cq/32
                nc.vector.tensor_scalar(
                    wdd_row[:, q_sl], ps1, 1.0 / 2048.0, cq_t,
                    MULT, ADD,
                )
            nc.gpsimd.partition_broadcast(wdd_b, wdd_row)

            # md tiles: rel DMA + mul, both on gpsimd (own queue, own engine)
            md_state = {"kc": 0}

            def emit_md_steps(n):
                for _ in range(n):
                    kc = md_state["kc"]
                    if kc >= KC:
                        return
                    md_state["kc"] += 1
                    rel_t = relp.tile([128, SQ], F8, tag="rel")
                    nc.gpsimd.dma_start(
                        rel_t, relT[kc * 128:(kc + 1) * 128, :])
                    nc.gpsimd.tensor_mul(md[:, kc, :], rel_t, wdd_b)
                    if not am_zero:
                        nc.gpsimd.tensor_scalar_add(
                            md[:, kc, :], md[:, kc, :], am_t[:, kc:kc + 1]
                        )

            # ---- v projection: v[s,d] for both halves ----
            for half, src in ((0, hsq_sb), (1, hso_sb)):
                for scl in range(KCH):
                    ps2 = psum_p.tile([128, 1024], F32, tag="pp")
                    for n0 in range(0, H, 512):
                        for dc in range(0, HC, 2):
                            nc.tensor.matmul(
                                ps2[:, n0:n0 + 512],
                                src[:, dc:dc + 2, scl * 128:(scl + 1) * 128],
                                wv_sb[:, dc:dc + 2, n0:n0 + 512],
                                start=(dc == 0),
                                stop=(dc == HC - 2),
                                perf_mode=DR,
                            )
                    nc.vector.tensor_scalar_mul(
                        v_sb[:, half * KCH + scl, :], ps2, 0.125
                    )
                    emit_md_steps(1)

            # ---- k projection -> kT[h, k] ; q projection -> qT[h, q] ----
            for w_d, bias_t, dst, srcs in (
                (wkT, bk_t, kT, ((0, hsq_sb), (1, hso_sb))),
                (wqT, bq_t, qT, ((0, hsq_sb),)),
            ):
                w_sb = wpool.tile([128, HC, H], F8, tag="w")
                nc.sync.dma_start(w_sb, w_d[:])
                for half, src in srcs:
                    for mc in range(HC):
                        ps2 = psum_p.tile([128, 1024], F32, tag="pp")
                        for n0 in range(0, SQ, 512):
                            for dc in range(0, HC, 2):
                                nc.tensor.matmul(
                                    ps2[:, n0:n0 + 512],
                                    w_sb[:, dc:dc + 2, mc * 128:(mc + 1) * 128],
                                    src[:, dc:dc + 2, n0:n0 + 512],
                                    start=(dc == 0),
                                    stop=(dc == HC - 2),
                                    perf_mode=DR,
                                )
                        col = half * SQ
                        # evacuate on ACT: dst = psum*0.125 + bias
                        nc.scalar.activation(
                            dst[:, mc, col:col + 1024], ps2,
                            AF.Identity,
                            bias=bias_t[:, mc:mc + 1],
                            scale=0.125,
                        )
                        emit_md_steps(1)
            emit_md_steps(KC)  # flush any remainder

        # ====== phase 2: scores (both q-halves at once) ======
        with (
            tc.tile_pool(name="wo_pool", bufs=1) as wo_pool,
            tc.tile_pool(name="hsq_pool", bufs=1) as hsq_pool,
            tc.tile_pool(name="epi", bufs=4) as epi,
            tc.tile_pool(name="denp", bufs=2) as denp,
            tc.tile_pool(name="stat", bufs=4) as stat,
        ):
            wo_sb = wo_pool.tile([128, HC, H], F8)
            nc.sync.dma_start(wo_sb, woT[:])
            hsq_sb2 = hsq_pool.tile([128, HC, H], BF16)
            nc.sync.dma_start(hsq_sb2, hsq[:])

            with (
                tc.tile_pool(name="smx", bufs=3) as smx,
                tc.tile_pool(name="psum_s", bufs=3, space="PSUM") as psum_s,
            ):
                for kc in range(KC):
                    ps2 = psum_s.tile([128, 1024], F32, tag="ps")
                    for q0 in range(0, SQ, 512):
                        for dc in range(0, HC, 2):
                            nc.tensor.matmul(
                                ps2[:, q0:q0 + 512],
                                kT[:, dc:dc + 2, kc * 128:(kc + 1) * 128],
                                qT[:, dc:dc + 2, q0:q0 + 512],
                                start=(dc == 0),
                                stop=(dc == HC - 2),
                                perf_mode=DR,
                            )
                    y_t = smx.tile([128, 1024], BF16, tag="y")
                    # y = ps/32 + md  (fused)
                    nc.vector.scalar_tensor_tensor(
                        y_t, ps2, INV_SQRT_DH, md[:, kc, :], MULT, ADD
                    )
                    nc.scalar.activation(expT[:, kc, :], y_t, AF.Exp)

                # denominators -> DRAM bounce -> [q-part, sc] -> reciprocal
                for qn in range(2):
                    q_sl = slice(qn * 512, (qn + 1) * 512)
                    dn = psum_sm.tile([1, 512], F32, tag="small")
                    for kc in range(0, KC, 2):
                        nc.tensor.matmul(
                            dn,
                            ones2,
                            expT[:, kc:kc + 2, q_sl],
                            start=(kc == 0),
                            stop=(kc == KC - 2),
                            perf_mode=DR,
                        )
                    dr_t = denp.tile([1, 512], F32, tag="dr")
                    nc.vector.tensor_copy(dr_t, dn)
                    nc.gpsimd.dma_start(den_d[qn * 512:(qn + 1) * 512], dr_t)
                    dcol = denp.tile([128, 4], F32, tag="dcol")
                    nc.gpsimd.dma_start(
                        dcol,
                        den_d[qn * 512:(qn + 1) * 512].rearrange(
                            "(s p) -> p s", p=128),
                    )
                    nc.vector.reciprocal(rinv[:, qn * 4:qn * 4 + 4], dcol)

            psum_pv = ctx.enter_context(
                tc.tile_pool(name="psum_pv", bufs=2, space="PSUM"))
            psum_ao = ctx.enter_context(
                tc.tile_pool(name="psum_ao", bufs=2, space="PSUM"))

            def pv_phase(qn):
                q_sl = slice(qn * 512, (qn + 1) * 512)
                for dc in range(HC):
                    pv = psum_pv.tile([128, 512], F32, tag="pv")
                    for kc in range(0, KC, 2):
                        nc.tensor.matmul(
                            pv,
                            v_sb[:, kc:kc + 2, dc * 128:(dc + 1) * 128],
                            expT[:, kc:kc + 2, q_sl],
                            start=(kc == 0),
                            stop=(kc == KC - 2),
                            perf_mode=DR,
                        )
                    nc.vector.tensor_scalar_mul(ctxT[:, dc, q_sl], pv, 0.125)

            def epilogue(qn):
                # out-proj (fp8 DR) + 1/den + residual + LN per 128-row chunk
                for sc in range(qn * 4, qn * 4 + 4):
                    ao2 = psum_ao.tile([128, 1024], F32, tag="ao")
                    for hn in range(2):
                        for dc in range(0, HC, 2):
                            nc.tensor.matmul(
                                ao2[:, hn * 512:(hn + 1) * 512],
                                ctxT[:, dc:dc + 2, sc * 128:(sc + 1) * 128],
                                wo_sb[:, dc:dc + 2, hn * 512:(hn + 1) * 512],
                                start=(dc == 0),
                                stop=(dc == HC - 2),
                                perf_mode=DR,
                            )
                    x_t = epi.tile([128, H], F32, tag="x")
                    xs2 = stat.tile([128, 2], F32, tag="xs")
                    x2s2 = stat.tile([128, 2], F32, tag="x2s")
                    sq_t = epi.tile([128, H], F32, tag="sq")
                    # halves pipeline the DVE->ACT chain (shorter tail)
                    for hn in range(2):
                        h_sl = slice(hn * 512, (hn + 1) * 512)
                        nc.vector.scalar_tensor_tensor(
                            x_t[:, h_sl], ao2[:, h_sl], rinv[:, sc:sc + 1],
                            hsq_sb2[:, sc, h_sl], MULT, ADD,
                            accum_out=xs2[:, hn:hn + 1],
                        )
                        nc.scalar.activation(
                            sq_t[:, h_sl], x_t[:, h_sl], AF.Square,
                            accum_out=x2s2[:, hn:hn + 1])
                    mean = stat.tile([128, 1], F32, tag="mu")
                    nc.vector.tensor_scalar(
                        mean, xs2[:, 0:1], xs2[:, 1:2], 1.0 / H, ADD, MULT)
                    ex2 = stat.tile([128, 1], F32, tag="ex2")
                    nc.vector.tensor_scalar(
                        ex2, x2s2[:, 0:1], x2s2[:, 1:2], 1.0 / H, ADD, MULT)
                    nmvar = stat.tile([128, 1], F32, tag="nv")
                    # nmvar = mean^2 - E[x^2] = -var
                    nc.vector.scalar_tensor_tensor(
                        nmvar, mean, mean, ex2, MULT,
                        mybir.AluOpType.subtract,
                    )
                    sd = stat.tile([128, 1], F32, tag="sd")
                    # sd = sqrt(-nmvar + eps)
                    nc.scalar.activation(
                        sd, nmvar, AF.Sqrt, bias=eps_t, scale=-1.0)
                    rq = stat.tile([128, 1], F32, tag="rq")
                    nc.vector.reciprocal(rq, sd)
                    nmur = stat.tile([128, 1], F32, tag="nm")
                    nc.vector.tensor_scalar(
                        nmur, mean, rq, -1.0, MULT, MULT
                    )
                    y_t = epi.tile([128, H], BF16, tag="yout")
                    nc.scalar.activation(
                        y_t[:, 0:512], x_t[:, 0:512], AF.Identity,
                        bias=nmur, scale=rq,
                    )
                    if ln_affine:
                        nc.vector.tensor_mul(
                            y_t[:, 0:512], y_t[:, 0:512], g_b[:, 0:512])
                        nc.vector.tensor_add(
                            y_t[:, 0:512], y_t[:, 0:512], b_b[:, 0:512])
                    nc.sync.dma_start(
                        out_d[sc * 128:(sc + 1) * 128, 0:512], y_t[:, 0:512])
                    # second half on DVE: y = (x - mu) * rq
                    nc.vector.tensor_scalar(
                        y_t[:, 512:1024], x_t[:, 512:1024], mean, rq,
                        mybir.AluOpType.subtract, MULT,
                    )
                    if ln_affine:
                        nc.vector.tensor_mul(
                            y_t[:, 512:1024], y_t[:, 512:1024], g_b[:, 512:1024])
                        nc.vector.tensor_add(
                            y_t[:, 512:1024], y_t[:, 512:1024], b_b[:, 512:1024])
                    nc.sync.dma_start(
                        out_d[sc * 128:(sc + 1) * 128, 512:1024],
                        y_t[:, 512:1024])

            pv_phase(0)
            pv_phase(1)
            epilogue(0)
            epilogue(1)

    nc.compile()
    return nc


def get_program(am_zero=True, ln_affine=False):
    key = ("nc", am_zero, ln_affine)
    if key not in _CACHE:
        _CACHE[key] = _build_program(am_zero, ln_affine)
    return _CACHE[key]


def make_in_maps(inputs, am_zero=None, ln_affine=None):
    """Host-side sharding / layout prep (numpy only)."""
    f32 = np.float32
    f8 = ml_dtypes.float8_e4m3
    hs = np.asarray(inputs["hidden_states"], dtype=f32)
    rel = np.asarray(inputs["word_word_relation"])
    am = np.asarray(inputs["attention_mask"], dtype=f32)  # [B,1,1,S]
    Wq = np.asarray(inputs["Wq"], dtype=f32)
    Wk = np.asarray(inputs["Wk"], dtype=f32)
    Wv = np.asarray(inputs["Wv"], dtype=f32)
    Wo = np.asarray(inputs["Wo"], dtype=f32)
    bq = np.asarray(inputs["bq"], dtype=f32)
    bk = np.asarray(inputs["bk"], dtype=f32)
    bv = np.asarray(inputs["bv"], dtype=f32)
    bo = np.asarray(inputs["bo"], dtype=f32)
    d1 = np.asarray(inputs["dist_emb"], dtype=f32)[1]
    lng = np.asarray(inputs["ln_g"], dtype=f32)
    lnb = np.asarray(inputs["ln_b"], dtype=f32)
    if am_zero is None:
        am_zero = bool(np.all(am == 0.0))
    if ln_affine is None:
        ln_affine = not (np.all(lng == 1.0) and np.all(lnb == 0.0))

    def sw(a):
        # [ (c p), m ] row-major -> [p, c, m] tile layout
        return np.ascontiguousarray(
            a.reshape(HC, 128, a.shape[-1]).transpose(1, 0, 2))

    wqT = sw((Wq.T * 8.0).astype(f8))
    wkT = sw((Wk.T * 8.0).astype(f8))
    wvT = sw((Wv.T * 8.0).astype(f8))
    woT = sw((Wo.T * 8.0).astype(f8))
    bo_eff = Wo @ bv + bo  # v/o biases fold into the residual
    bq_t = np.ascontiguousarray(bq.reshape(HC, 128).T)
    bk_t = np.ascontiguousarray(bk.reshape(HC, 128).T)
    u = (Wq.astype(np.float64).T @ d1.astype(np.float64)).astype(f32)
    u_t = np.zeros((128, HC, 64), dtype=f8)
    u_t[:, :, 0] = ((u * 64.0).reshape(HC, 128).T).astype(f8)
    cq32 = np.array([[float(bq @ d1) / 32.0]], dtype=f32)
    relm = (rel == 1)

    in_maps = []
    for c in range(N_CORES):
        b, qh = divmod(c, 2)
        qs = qh * SQ
        os_ = (1 - qh) * SQ
        kidx = np.r_[qs:qs + SQ, os_:os_ + SQ]   # [own keys, other keys]
        m = {
            "hsqT": sw(np.ascontiguousarray(hs[b, qs:qs + SQ, :].T).astype(f8)),
            "hsoT": sw(np.ascontiguousarray(hs[b, os_:os_ + SQ, :].T).astype(f8)),
            "hsq": sw((hs[b, qs:qs + SQ, :] + bo_eff).astype(ml_dtypes.bfloat16)),
            "relT": np.ascontiguousarray(
                relm[b, qs:qs + SQ, :].T[kidx, :]).astype(f8),
            "wqT": wqT, "wkT": wkT, "wvT": wvT, "woT": woT,
            "u": u_t, "cq32": cq32, "bq": bq_t, "bk": bk_t,
        }
        if not am_zero:
            m["am"] = np.ascontiguousarray(
                am[b, 0, 0][kidx].reshape(KC, 128).T)
        if ln_affine:
            m["lng"] = lng
            m["lnb"] = lnb
        in_maps.append(m)
    return in_maps


def kernel(**inputs):
    am = np.asarray(inputs["attention_mask"], dtype=np.float32)
    am_zero = bool(np.all(am == 0.0))
    ln_affine = not (
        np.all(np.asarray(inputs["ln_g"]) == 1.0)
        and np.all(np.asarray(inputs["ln_b"]) == 0.0)
    )
    nc = get_program(am_zero, ln_affine)
    in_maps = make_in_maps(inputs, am_zero, ln_affine)
    res = run_bass_kernel_spmd(nc, in_maps, core_ids=list(range(N_CORES)))
    out = np.empty((B, S, H), dtype=np.float32)
    for c in range(N_CORES):
        b, qh = divmod(c, 2)
        out[b, qh * SQ:(qh + 1) * SQ, :] = res.results[c]["out"]
    return out


# revision 23
# speedup vs baseline: 1.0148x; 1.0148x over previous
"""Trainium2 Bass kernel for nn_BertAttentionDistance (B=4, S=2048, H=1024, NH=1, DT=32).

Sharding: 8 cores = (batch b = c//2) x (query-half qh = c%2, 1024 q-rows each).
K/V projection work for a batch is replicated across its 2 cores (no collectives).
Key order per core is [own 1024 keys, other 1024 keys]; relT/am are permuted to
match on the host (softmax/PV are order-invariant over keys).

Math notes (exact identities w.r.t. the reference):
  * take_along_axis(word_dot_distance, rel, 3) * (rel == 1)
      == (q . dist_emb[1]) * (rel == 1)           (gather collapses)
  * wdd1 = q . d1 = hs . (Wq^T d1) + bq . d1 = hs . u + cq  (u, cq on host)
  * softmax max-subtraction skipped: scores/32 is O(+-3), safely in fp32 exp range.
  * v-bias and o-bias fold into the residual: x = Wo@ctx + (Wo@bv + bo) + hs,
      folded on host into hsq' = hs_q + Wo@bv + bo.
  * attention_mask folds into md = rel*wdd1/32 + am_k (general variant only).
  * ctxT stays UNNORMALIZED (pv/8 in fp8); 1/denominator is applied in the
      epilogue as a per-partition (per-query-row) scalar on the x-assembly,
      using denominators transposed via a DRAM bounce.

Engine split: PE all matmuls (fp8 DR); DVE psum evacuations (v, scores-stt,
pv, epi-stt) + LN stats; ACT k/q evacuations + exp + LN normalize; GPSIMD
md = rel*(wdd1/32) muls + rel DMA queue; sync queue big streams.
"""

import sys

for p in ("/opt/trn_rl_repo", "/opt/pypackages"):
    if p not in sys.path:
        sys.path.insert(0, p)

from contextlib import ExitStack

import ml_dtypes
import numpy as np

import concourse.bacc as bacc
import concourse.bass as bass
import concourse.tile as tile
from concourse import mybir
from concourse.bass_utils import run_bass_kernel_spmd

# Problem constants (hardcoded per the harness contract).
B, S, H = 4, 2048, 1024
N_CORES = 8
SQ = 1024          # query rows per core
HC = H // 128      # 8 chunks of 128 over hidden/head dim
KC = S // 128      # 16 chunks of 128 over key dim
KCH = KC // 2      # key chunks per half
LN_EPS = 1e-12
INV_SQRT_DH = 1.0 / 32.0

F32 = mybir.dt.float32
BF16 = mybir.dt.bfloat16
F8 = mybir.dt.float8e4
DR = mybir.MatmulPerfMode.DoubleRow
MULT = mybir.AluOpType.mult
ADD = mybir.AluOpType.add
AF = mybir.ActivationFunctionType

_CACHE = {}


def _build_program(am_zero=True, ln_affine=False):
    nc = bacc.Bacc("TRN2", target_bir_lowering=False, debug=False)

    # All big inputs are pre-swizzled on the host to exact SBUF tile layout
    # ([128 partitions, chunk, free]) so each DMA is 128 large descriptors.
    hsqT = nc.dram_tensor("hsqT", [128, HC, SQ], F8, kind="ExternalInput")
    hsoT = nc.dram_tensor("hsoT", [128, HC, SQ], F8, kind="ExternalInput")
    hsq = nc.dram_tensor("hsq", [128, HC, H], BF16, kind="ExternalInput")
    relT = nc.dram_tensor("relT", [S, SQ], F8, kind="ExternalInput")
    wqT = nc.dram_tensor("wqT", [128, HC, H], F8, kind="ExternalInput")
    wkT = nc.dram_tensor("wkT", [128, HC, H], F8, kind="ExternalInput")
    wvT = nc.dram_tensor("wvT", [128, HC, H], F8, kind="ExternalInput")
    woT = nc.dram_tensor("woT", [128, HC, H], F8, kind="ExternalInput")
    u_d = nc.dram_tensor("u", [128, HC, 64], F8, kind="ExternalInput")
    cq_d = nc.dram_tensor("cq32", [1, 1], F32, kind="ExternalInput")
    bq_d = nc.dram_tensor("bq", [128, HC], F32, kind="ExternalInput")
    bk_d = nc.dram_tensor("bk", [128, HC], F32, kind="ExternalInput")
    if not am_zero:
        am_d = nc.dram_tensor("am", [128, KC], F32, kind="ExternalInput")
    if ln_affine:
        lng_d = nc.dram_tensor("lng", [H], F32, kind="ExternalInput")
        lnb_d = nc.dram_tensor("lnb", [H], F32, kind="ExternalInput")
    den_d = nc.dram_tensor("den_scratch", [SQ], F32, kind="Internal")
    out_d = nc.dram_tensor("out", [SQ, H], BF16, kind="ExternalOutput")

    def bcast_rows(src_1d_ap, p=128):
        return bass.AP(
            tensor=src_1d_ap.tensor,
            offset=src_1d_ap.offset,
            ap=[[0, p], *src_1d_ap.ap],
        )

    with tile.TileContext(nc) as tc, ExitStack() as ctx:
        consts = ctx.enter_context(tc.tile_pool(name="consts", bufs=1))
        big = ctx.enter_context(tc.tile_pool(name="big", bufs=1))
        psum_sm = ctx.enter_context(tc.tile_pool(name="psum_sm", bufs=2, space="PSUM"))

        # DR ldweights needs >=64B stride between the two k-tiles: pad dim2.
        ones2_pad = consts.tile([128, 2, 64], F8)
        nc.vector.memset(ones2_pad, 1.0)
        ones2 = ones2_pad[:, :, 0:1]
        eps_t = consts.tile([128, 1], F32)
        nc.vector.memset(eps_t, LN_EPS)
        u_pad = consts.tile([128, HC, 64], F8)
        u_t = u_pad[:, :, 0:1]

        cq_t = consts.tile([1, 1], F32)
        bq_t = consts.tile([128, HC], F32)
        bk_t = consts.tile([128, HC], F32)
        if not am_zero:
            am_t = consts.tile([128, KC], F32)
            nc.gpsimd.dma_start(am_t, am_d[:])
        if ln_affine:
            g_b = consts.tile([128, H], F32)
            nc.gpsimd.dma_start(g_b, bcast_rows(lng_d[:]))
            b_b = consts.tile([128, H], F32)
            nc.gpsimd.dma_start(b_b, bcast_rows(lnb_d[:]))

        # ---- persistent big tensors ----
        kT = big.tile([128, HC, S], F8)        # [d-part, dc, k(own,other)]
        v_sb = big.tile([128, KC, H], F8)      # [s-part(own,other), kc, d]
        qT = big.tile([128, HC, SQ], F8)       # [d-part, dc, q]
        ctxT = big.tile([128, HC, SQ], F8)     # [d-part, dc, q] = pv/8
        md = big.tile([128, KC, SQ], BF16)     # rel*(wdd1/32) (+am) [k-part, kc, q]
        wdd_b = big.tile([128, SQ], BF16)      # broadcast of wdd1/32
        wdd_row = big.tile([1, SQ], BF16)
        expT = big.tile([128, KC, SQ], F8)     # exp (unnormalized) [k-part, kc, q]
        rinv = big.tile([128, HC], F32)        # 1/denom, [q-within-sc, sc]

        # ================= phase 1: projections + md =================
        with (
            tc.tile_pool(name="hs_pool", bufs=1) as hs_pool,
            tc.tile_pool(name="wpool", bufs=2) as wpool,
            tc.tile_pool(name="relp", bufs=3) as relp,
            tc.tile_pool(name="psum_p", bufs=3, space="PSUM") as psum_p,
        ):
            hsq_sb = hs_pool.tile([128, HC, SQ], F8)   # own rows (q==own keys)
            hso_sb = hs_pool.tile([128, HC, SQ], F8)   # other-half key rows

            wv_sb = wpool.tile([128, HC, H], F8, tag="w")

            # sync-queue DMA order = need order: u/cq first (wdd), then the
            # big streams; biases before the k/q evacuations need them.
            nc.sync.dma_start(u_pad, u_d[:])
            nc.sync.dma_start(cq_t, cq_d[:])
            nc.sync.dma_start(hsq_sb, hsqT[:])
            nc.sync.dma_start(wv_sb, wvT[:])
            nc.gpsimd.dma_start(hso_sb, hsoT[:])
            nc.sync.dma_start(bq_t, bq_d[:])
            nc.sync.dma_start(bk_t, bk_d[:])

            # ---- wdd1/32 row: M=1 DR matmuls from hsqT ----
            for qn in range(2):
                q_sl = slice(qn * 512, (qn + 1) * 512)
                ps1 = psum_sm.tile([1, 512], F32, tag="small")
                for dc in range(0, HC, 2):
                    nc.tensor.matmul(
                        ps1,
                        u_t[:, dc:dc + 2, :],
                        hsq_sb[:, dc:dc + 2, q_sl],
                        start=(dc == 0),
                        stop=(dc == HC - 2),
                        perf_mode=DR,
                    )
                # wdd1/32 = psum/(64*32) + cq/32
                nc.vector.tensor_scalar(
                    wdd_row[:, q_sl], ps1, 1.0 / 2048.0, cq_t,
                    MULT, ADD,
                )
            nc.gpsimd.partition_broadcast(wdd_b, wdd_row)

            # md tiles: rel DMA + mul, both on gpsimd (own queue, own engine)
            md_state = {"kc": 0}

            def emit_md_steps(n):
                for _ in range(n):
                    kc = md_state["kc"]
                    if kc >= KC:
                        return
                    md_state["kc"] += 1
                    rel_t = relp.tile([128, SQ], F8, tag="rel")
                    nc.gpsimd.dma_start(
                        rel_t, relT[kc * 128:(kc + 1) * 128, :])
                    nc.gpsimd.tensor_mul(md[:, kc, :], rel_t, wdd_b)
                    if not am_zero:
                        nc.gpsimd.tensor_scalar_add(
                            md[:, kc, :], md[:, kc, :], am_t[:, kc:kc + 1]
                        )

            # ---- v projection: v[s,d] for both halves ----
            for half, src in ((0, hsq_sb), (1, hso_sb)):
                for scl in range(KCH):
                    ps2 = psum_p.tile([128, 1024], F32, tag="pp")
                    for n0 in range(0, H, 512):
                        for dc in range(0, HC, 2):
                            nc.tensor.matmul(
                                ps2[:, n0:n0 + 512],
                                src[:, dc:dc + 2, scl * 128:(scl + 1) * 128],
                                wv_sb[:, dc:dc + 2, n0:n0 + 512],
                                start=(dc == 0),
                                stop=(dc == HC - 2),
                                perf_mode=DR,
                            )
                    nc.vector.tensor_scalar_mul(
                        v_sb[:, half * KCH + scl, :], ps2, 0.125
                    )
                    emit_md_steps(1)

            # ---- k projection -> kT[h, k] ; q projection -> qT[h, q] ----
            for w_d, bias_t, dst, srcs in (
                (wkT, bk_t, kT, ((0, hsq_sb), (1, hso_sb))),
                (wqT, bq_t, qT, ((0, hsq_sb),)),
            ):
                w_sb = wpool.tile([128, HC, H], F8, tag="w")
                nc.sync.dma_start(w_sb, w_d[:])
                for half, src in srcs:
                    for mc in range(HC):
                        ps2 = psum_p.tile([128, 1024], F32, tag="pp")
                        for n0 in range(0, SQ, 512):
                            for dc in range(0, HC, 2):
                                nc.tensor.matmul(
                                    ps2[:, n0:n0 + 512],
                                    w_sb[:, dc:dc + 2, mc * 128:(mc + 1) * 128],
                                    src[:, dc:dc + 2, n0:n0 + 512],
                                    start=(dc == 0),
                                    stop=(dc == HC - 2),
                                    perf_mode=DR,
                                )
                        col = half * SQ
                        # evacuate on ACT: dst = psum*0.125 + bias
                        nc.scalar.activation(
                            dst[:, mc, col:col + 1024], ps2,
                            AF.Identity,
                            bias=bias_t[:, mc:mc + 1],
                            scale=0.125,
                        )
                        emit_md_steps(1)
            emit_md_steps(KC)  # flush any remainder

        # ====== phase 2: scores (both q-halves at once) ======
        with (
            tc.tile_pool(name="wo_pool", bufs=1) as wo_pool,
            tc.tile_pool(name="hsq_pool", bufs=1) as hsq_pool,
            tc.tile_pool(name="epi", bufs=4) as epi,
            tc.tile_pool(name="denp", bufs=2) as denp,
            tc.tile_pool(name="stat", bufs=4) as stat,
        ):
            wo_sb = wo_pool.tile([128, HC, H], F8)
            nc.sync.dma_start(wo_sb, woT[:])
            hsq_sb2 = hsq_pool.tile([128, HC, H], BF16)
            nc.sync.dma_start(hsq_sb2, hsq[:])

            with (
                tc.tile_pool(name="smx", bufs=3) as smx,
                tc.tile_pool(name="psum_s", bufs=3, space="PSUM") as psum_s,
            ):
                for kc in range(KC):
                    ps2 = psum_s.tile([128, 1024], F32, tag="ps")
                    for q0 in range(0, SQ, 512):
                        for dc in range(0, HC, 2):
                            nc.tensor.matmul(
                                ps2[:, q0:q0 + 512],
                                kT[:, dc:dc + 2, kc * 128:(kc + 1) * 128],
                                qT[:, dc:dc + 2, q0:q0 + 512],
                                start=(dc == 0),
                                stop=(dc == HC - 2),
                                perf_mode=DR,
                            )
                    y_t = smx.tile([128, 1024], BF16, tag="y")
                    # y = ps/32 + md  (fused)
                    nc.vector.scalar_tensor_tensor(
                        y_t, ps2, INV_SQRT_DH, md[:, kc, :], MULT, ADD
                    )
                    nc.scalar.activation(expT[:, kc, :], y_t, AF.Exp)

                # denominators -> DRAM bounce -> [q-part, sc] -> reciprocal
                for qn in range(2):
                    q_sl = slice(qn * 512, (qn + 1) * 512)
                    dn = psum_sm.tile([1, 512], F32, tag="small")
                    for kc in range(0, KC, 2):
                        nc.tensor.matmul(
                            dn,
                            ones2,
                            expT[:, kc:kc + 2, q_sl],
                            start=(kc == 0),
                            stop=(kc == KC - 2),
                            perf_mode=DR,
                        )
                    dr_t = denp.tile([1, 512], F32, tag="dr")
                    nc.vector.tensor_copy(dr_t, dn)
                    nc.gpsimd.dma_start(den_d[qn * 512:(qn + 1) * 512], dr_t)
                    dcol = denp.tile([128, 4], F32, tag="dcol")
                    nc.gpsimd.dma_start(
                        dcol,
                        den_d[qn * 512:(qn + 1) * 512].rearrange(
                            "(s p) -> p s", p=128),
                    )
                    nc.vector.reciprocal(rinv[:, qn * 4:qn * 4 + 4], dcol)

            psum_pv = ctx.enter_context(
                tc.tile_pool(name="psum_pv", bufs=2, space="PSUM"))
            psum_ao = ctx.enter_context(
                tc.tile_pool(name="psum_ao", bufs=2, space="PSUM"))

            def pv_phase(qn):
                q_sl = slice(qn * 512, (qn + 1) * 512)
                for dc in range(HC):
                    pv = psum_pv.tile([128, 512], F32, tag="pv")
                    for kc in range(0, KC, 2):
                        nc.tensor.matmul(
                            pv,
                            v_sb[:, kc:kc + 2, dc * 128:(dc + 1) * 128],
                            expT[:, kc:kc + 2, q_sl],
                            start=(kc == 0),
                            stop=(kc == KC - 2),
                            perf_mode=DR,
                        )
                    nc.vector.tensor_scalar_mul(ctxT[:, dc, q_sl], pv, 0.125)

            def epilogue(qn):
                # out-proj (fp8 DR) + 1/den + residual + LN per 128-row chunk
                for sc in range(qn * 4, qn * 4 + 4):
                    ao2 = psum_ao.tile([128, 1024], F32, tag="ao")
                    for hn in range(2):
                        for dc in range(0, HC, 2):
                            nc.tensor.matmul(
                                ao2[:, hn * 512:(hn + 1) * 512],
                                ctxT[:, dc:dc + 2, sc * 128:(sc + 1) * 128],
                                wo_sb[:, dc:dc + 2, hn * 512:(hn + 1) * 512],
                                start=(dc == 0),
                                stop=(dc == HC - 2),
                                perf_mode=DR,
                            )
                    x_t = epi.tile([128, H], F32, tag="x")
                    xs2 = stat.tile([128, 2], F32, tag="xs")
                    x2s2 = stat.tile([128, 2], F32, tag="x2s")
                    sq_t = epi.tile([128, H], F32, tag="sq")
                    # halves pipeline the DVE->ACT chain (shorter tail)
                    for hn in range(2):
                        h_sl = slice(hn * 512, (hn + 1) * 512)
                        nc.vector.scalar_tensor_tensor(
                            x_t[:, h_sl], ao2[:, h_sl], rinv[:, sc:sc + 1],
                            hsq_sb2[:, sc, h_sl], MULT, ADD,
                            accum_out=xs2[:, hn:hn + 1],
                        )
                        nc.scalar.activation(
                            sq_t[:, h_sl], x_t[:, h_sl], AF.Square,
                            accum_out=x2s2[:, hn:hn + 1])
                    mean = stat.tile([128, 1], F32, tag="mu")
                    nc.vector.tensor_scalar(
                        mean, xs2[:, 0:1], xs2[:, 1:2], 1.0 / H, ADD, MULT)
                    ex2 = stat.tile([128, 1], F32, tag="ex2")
                    nc.vector.tensor_scalar(
                        ex2, x2s2[:, 0:1], x2s2[:, 1:2], 1.0 / H, ADD, MULT)
                    nmvar = stat.tile([128, 1], F32, tag="nv")
                    # nmvar = mean^2 - E[x^2] = -var
                    nc.vector.scalar_tensor_tensor(
                        nmvar, mean, mean, ex2, MULT,
                        mybir.AluOpType.subtract,
                    )
                    sd = stat.tile([128, 1], F32, tag="sd")
                    # sd = sqrt(-nmvar + eps)
                    nc.scalar.activation(
                        sd, nmvar, AF.Sqrt, bias=eps_t, scale=-1.0)
                    rq = stat.tile([128, 1], F32, tag="rq")
                    nc.vector.reciprocal(rq, sd)
                    nmur = stat.tile([128, 1], F32, tag="nm")
                    nc.vector.tensor_scalar(
                        nmur, mean, rq, -1.0, MULT, MULT
                    )
                    y_t = epi.tile([128, H], BF16, tag="yout")
                    nc.scalar.activation(
                        y_t[:, 0:512], x_t[:, 0:512], AF.Identity,
                        bias=nmur, scale=rq,
                    )
                    if ln_affine:
                        nc.vector.tensor_mul(
                            y_t[:, 0:512], y_t[:, 0:512], g_b[:, 0:512])
                        nc.vector.tensor_add(
                            y_t[:, 0:512], y_t[:, 0:512], b_b[:, 0:512])
                    nc.sync.dma_start(
                        out_d[sc * 128:(sc + 1) * 128, 0:512], y_t[:, 0:512])
                    # second half on DVE: y = (x - mu) * rq
                    nc.vector.tensor_scalar(
                        y_t[:, 512:1024], x_t[:, 512:1024], mean, rq,
                        mybir.AluOpType.subtract, MULT,
                    )
                    if ln_affine:
                        nc.vector.tensor_mul(
                            y_t[:, 512:1024], y_t[:, 512:1024], g_b[:, 512:1024])
                        nc.vector.tensor_add(
                            y_t[:, 512:1024], y_t[:, 512:1024], b_b[:, 512:1024])
                    nc.sync.dma_start(
                        out_d[sc * 128:(sc + 1) * 128, 512:1024],
                        y_t[:, 512:1024])

            pv_phase(0)
            epilogue(0)
            pv_phase(1)
            epilogue(1)

    nc.compile()
    return nc


def get_program(am_zero=True, ln_affine=False):
    key = ("nc", am_zero, ln_affine)
    if key not in _CACHE:
        _CACHE[key] = _build_program(am_zero, ln_affine)
    return _CACHE[key]


def make_in_maps(inputs, am_zero=None, ln_affine=None):
    """Host-side sharding / layout prep (numpy only)."""
    f32 = np.float32
    f8 = ml_dtypes.float8_e4m3
    hs = np.asarray(inputs["hidden_states"], dtype=f32)
    rel = np.asarray(inputs["word_word_relation"])
    am = np.asarray(inputs["attention_mask"], dtype=f32)  # [B,1,1,S]
    Wq = np.asarray(inputs["Wq"], dtype=f32)
    Wk = np.asarray(inputs["Wk"], dtype=f32)
    Wv = np.asarray(inputs["Wv"], dtype=f32)
    Wo = np.asarray(inputs["Wo"], dtype=f32)
    bq = np.asarray(inputs["bq"], dtype=f32)
    bk = np.asarray(inputs["bk"], dtype=f32)
    bv = np.asarray(inputs["bv"], dtype=f32)
    bo = np.asarray(inputs["bo"], dtype=f32)
    d1 = np.asarray(inputs["dist_emb"], dtype=f32)[1]
    lng = np.asarray(inputs["ln_g"], dtype=f32)
    lnb = np.asarray(inputs["ln_b"], dtype=f32)
    if am_zero is None:
        am_zero = bool(np.all(am == 0.0))
    if ln_affine is None:
        ln_affine = not (np.all(lng == 1.0) and np.all(lnb == 0.0))

    def sw(a):
        # [ (c p), m ] row-major -> [p, c, m] tile layout
        return np.ascontiguousarray(
            a.reshape(HC, 128, a.shape[-1]).transpose(1, 0, 2))

    wqT = sw((Wq.T * 8.0).astype(f8))
    wkT = sw((Wk.T * 8.0).astype(f8))
    wvT = sw((Wv.T * 8.0).astype(f8))
    woT = sw((Wo.T * 8.0).astype(f8))
    bo_eff = Wo @ bv + bo  # v/o biases fold into the residual
    bq_t = np.ascontiguousarray(bq.reshape(HC, 128).T)
    bk_t = np.ascontiguousarray(bk.reshape(HC, 128).T)
    u = (Wq.astype(np.float64).T @ d1.astype(np.float64)).astype(f32)
    u_t = np.zeros((128, HC, 64), dtype=f8)
    u_t[:, :, 0] = ((u * 64.0).reshape(HC, 128).T).astype(f8)
    cq32 = np.array([[float(bq @ d1) / 32.0]], dtype=f32)
    relm = (rel == 1)

    in_maps = []
    for c in range(N_CORES):
        b, qh = divmod(c, 2)
        qs = qh * SQ
        os_ = (1 - qh) * SQ
        kidx = np.r_[qs:qs + SQ, os_:os_ + SQ]   # [own keys, other keys]
        m = {
            "hsqT": sw(np.ascontiguousarray(hs[b, qs:qs + SQ, :].T).astype(f8)),
            "hsoT": sw(np.ascontiguousarray(hs[b, os_:os_ + SQ, :].T).astype(f8)),
            "hsq": sw((hs[b, qs:qs + SQ, :] + bo_eff).astype(ml_dtypes.bfloat16)),
            "relT": np.ascontiguousarray(
                relm[b, qs:qs + SQ, :].T[kidx, :]).astype(f8),
            "wqT": wqT, "wkT": wkT, "wvT": wvT, "woT": woT,
            "u": u_t, "cq32": cq32, "bq": bq_t, "bk": bk_t,
        }
        if not am_zero:
            m["am"] = np.ascontiguousarray(
                am[b, 0, 0][kidx].reshape(KC, 128).T)
        if ln_affine:
            m["lng"] = lng
            m["lnb"] = lnb
        in_maps.append(m)
    return in_maps


def kernel(**inputs):
    am = np.asarray(inputs["attention_mask"], dtype=np.float32)
    am_zero = bool(np.all(am == 0.0))
    ln_affine = not (
        np.all(np.asarray(inputs["ln_g"]) == 1.0)
        and np.all(np.asarray(inputs["ln_b"]) == 0.0)
    )
    nc = get_program(am_zero, ln_affine)
    in_maps = make_in_maps(inputs, am_zero, ln_affine)
    res = run_bass_kernel_spmd(nc, in_maps, core_ids=list(range(N_CORES)))
    out = np.empty((B, S, H), dtype=np.float32)
    for c in range(N_CORES):
        b, qh = divmod(c, 2)
        out[b, qh * SQ:(qh + 1) * SQ, :] = res.results[c]["out"]
    return out
